# revision 2
# baseline (speedup 1.0000x reference)
"""Trainium2 Bass kernel for AssetGATEncoder (2-layer GATv2, N=30000, E=480000).

v2 strategy (8 NeuronCores, SPMD, dst-partitioned):
- Nodes partitioned by DESTINATION: core c owns dst rows [c*3750, (c+1)*3750),
  30 blocks of 128 dst rows each. Host sorts edges (with self-loops) by dst.
- Per layer each core computes its local xl table rows (xl = h@Wl + bl),
  AllGathers ONLY the xl table (bf16, Shared output); xr rows stay local
  (xr of the current 128 dst rows is all the xr the block ever needs).
- Per block: ONE dma_gather of xl[src] rows (trailing -1 indices skip
  transfers for padding), then
    u = xl[src] + xr[dst]  via PE: ident-inject matmul + one-hot "maskT"
        expansion matmul accumulating into the same PSUM tile,
    z = leaky_relu(u)      via Scalar ACT (PSUM->SBUF, alpha=0.2),
    e = sum_k att_k*z_k    via DVE (in-place z*att, 4D tensor_reduce),
    a = exp(e-3)           via Scalar,
    wa = [a*xl | a]        via DVE,
    per-dst softmax sums + messages via one-hot masked matmul on PE.
- Whole kernel uses a single activation table set (exp/ln/lrelu):
  LN rsqrt is computed as exp(-0.5*ln(var+eps)).
"""
import os
import numpy as np
import ml_dtypes

import concourse.bacc as bacc
import concourse.bass as bass
import concourse.mybir as mybir
import concourse.tile as tile
from concourse.bass_utils import run_bass_kernel_spmd

F32 = mybir.dt.float32
BF16 = mybir.dt.bfloat16
FP8 = mybir.dt.float8e4
I16 = mybir.dt.int16
AF = mybir.ActivationFunctionType
OP = mybir.AluOpType

N = 30000
NCORES = 8
NLOC = N // NCORES          # 3750 dst nodes per core
NBLK = (NLOC + 127) // 128  # 30 blocks of 128 dsts
F_IN = 128
HID = 64
HEADS = 4
FEAT0 = HEADS * HID         # 256
EMB = 32
SLOPE = 0.2
SM_EPS = 1e-16
LN_EPS = 1e-5
EXP_SHIFT = -3.0
# exact lrelu(0.2) from plain relu (relu is in every act table set, so no
# table reloads): lrelu02(x) = 0.2*x + relu(0.8*x)
A_LR = 1.0 - SLOPE
B_LR = SLOPE

LAST_EXEC_NS = None
LAST_RESULT = None
bf = ml_dtypes.bfloat16
f8 = ml_dtypes.float8_e4m3


def _wrap_idx(idx, eb):
    """[eb] int -> [128, eb//16] int16 wrapped layout for dma_gather."""
    a = idx.reshape(eb // 16, 16).T.astype(np.int16)   # [16, eb/16]
    return np.tile(a, (8, 1))                          # [128, eb/16]


def _bcast(ap, extra):
    """Append zero-stride dims to an AP: extra = list of counts."""
    return bass.AP(tensor=ap.tensor, offset=ap.offset,
                   ap=[*ap.ap, *[[0, c] for c in extra]])


def _mid_bcast(ap, count):
    """[p, X] AP -> [p, count(bcast), X]."""
    return bass.AP(tensor=ap.tensor, offset=ap.offset,
                   ap=[ap.ap[0], [0, count], *ap.ap[1:]])


def _host_prep(x, edge_index, Wp, bp, Wl0, bl0, Wr0, br0, att0, bias0, g0, be0,
               Wl1, bl1, Wr1, br1, att1, bias1, g1, be1):
    src = np.asarray(edge_index[0], np.int64)
    dst = np.asarray(edge_index[1], np.int64)
    si = np.arange(N, dtype=np.int64)
    src = np.concatenate([src, si])
    dst = np.concatenate([dst, si])
    order = np.argsort(dst, kind="stable")
    src, dst = src[order], dst[order]

    # bucket edges per core / per 128-dst block
    per_core = []
    eb_max = 0
    for c in range(NCORES):
        lo, hi = c * NLOC, (c + 1) * NLOC
        m = (dst >= lo) & (dst < hi)
        s_c, d_c = src[m], dst[m] - lo
        blocks = []
        for b in range(NBLK):
            mb = (d_c >= b * 128) & (d_c < min((b + 1) * 128, NLOC))
            blocks.append((s_c[mb], d_c[mb] - b * 128))
            eb_max = max(eb_max, mb.sum())
        per_core.append(blocks)
    eb = int(np.ceil(eb_max / 128) * 128)
    nch = eb // 128
    # per-block active chunk count (max across cores; program is SPMD)
    ncb = [max(int(np.ceil(max(len(per_core[c][b][0]), 1) / 128))
               for c in range(NCORES)) for b in range(NBLK)]
    offs = np.concatenate([[0], np.cumsum(ncb)]).astype(int)
    TOT = int(offs[-1])

    sidx = np.full((NCORES, NBLK, 128, eb // 16), -1, np.int16)
    masks = np.zeros((NCORES, 128, TOT * 128), f8)     # edge-major
    masksT = np.zeros((NCORES, 128, TOT * 128), f8)    # dst-major
    for c in range(NCORES):
        for b in range(NBLK):
            s_b, dloc = per_core[c][b]
            ne = len(s_b)
            nb = ncb[b]
            sp = np.zeros(eb, np.int64)
            sp[:ne] = s_b
            sidx[c, b] = _wrap_idx(sp, eb)
            mm = np.zeros((nb * 128, 128), np.float32)
            mm[np.arange(ne), dloc] = 1.0
            m3 = mm.reshape(nb, 128, 128)
            mb_e = m3.transpose(1, 0, 2).reshape(128, nb * 128)
            mb_d = m3.transpose(2, 0, 1).reshape(128, nb * 128)
            masks[c, :, offs[b] * 128:offs[b + 1] * 128] = mb_e.astype(f8)
            masksT[c, :, offs[b] * 128:offs[b + 1] * 128] = mb_d.astype(f8)

    xT = np.ascontiguousarray(np.asarray(x, np.float32).T.astype(bf))  # [128, N]

    def t128(v, w):  # replicate a row vector to a [128, w] tile
        return np.tile(np.asarray(v, np.float32).reshape(1, w), (128, 1))

    W1 = np.concatenate([np.asarray(Wl1), np.asarray(Wr1)], 1)  # [256, 64]
    consts = {
        "Wp": np.asarray(Wp, np.float32).astype(bf),                   # [128,64]
        "bp_t": t128(bp, HID).astype(np.float32),                      # [128,64]
        "Wl0": np.asarray(Wl0, np.float32).astype(bf),                 # [64,256]
        "Wr0": np.asarray(Wr0, np.float32).astype(bf),                 # [64,256]
        "bl0_t": t128(bl0, FEAT0).astype(bf),                          # [128,256]
        "br0_t": t128(br0, FEAT0).astype(bf),                          # [128,256]
        "W1h": np.concatenate([W1[:128], W1[128:]], 1).astype(bf),     # [128,128]
        "bpk1_t": t128(np.concatenate([bl1, br1]), 2 * EMB).astype(bf),
        "att0_t": t128(np.asarray(att0).reshape(-1), FEAT0).astype(bf),
        "att1_t": t128(np.asarray(att1).reshape(-1), EMB).astype(bf),
        "bga0_t": t128(bias0, FEAT0).astype(bf),                       # [128,256]
        "bga1_t": t128(bias1, EMB).astype(bf),
        "g0_t": t128(g0, FEAT0).astype(bf),
        "be0_t": t128(be0, FEAT0).astype(bf),
        "g1_t": t128(g1, EMB).astype(bf),
        "be1_t": t128(be1, EMB).astype(bf),
        "ident": np.eye(128).astype(bf),
    }
    return xT, sidx, masks, masksT, consts, eb, nch, ncb, list(offs)


def _build(eb, nch, ncb, offs):
    TOTCH = offs[-1]
    NQ = max(1, min(4, int(os.environ.get("K_NQ", "4"))))
    SCR = int(os.environ.get("K_SCR", "16384"))
    nc = bacc.Bacc("TRN2", target_bir_lowering=False, num_swdge_queues=NQ,
                   dynamic_dma_scratch_size=SCR)

    # ---- external inputs
    P = {}
    for name, shape, dt in [
        ("xT", [F_IN, NLOC], BF16),
        ("sidx", [NBLK, 128, eb // 16], I16),
        ("masks", [128, TOTCH * 128], FP8),
        ("masksT", [128, TOTCH * 128], FP8),
        ("Wp", [F_IN, HID], BF16), ("bp_t", [128, HID], F32),
        ("Wl0", [HID, FEAT0], BF16), ("Wr0", [HID, FEAT0], BF16),
        ("bl0_t", [128, FEAT0], BF16), ("br0_t", [128, FEAT0], BF16),
        ("W1h", [128, 4 * EMB], BF16), ("bpk1_t", [128, 2 * EMB], BF16),
        ("att0_t", [128, FEAT0], BF16), ("att1_t", [128, EMB], BF16),
        ("bga0_t", [128, FEAT0], BF16), ("bga1_t", [128, EMB], BF16),
        ("g0_t", [128, FEAT0], BF16), ("be0_t", [128, FEAT0], BF16),
        ("g1_t", [128, EMB], BF16), ("be1_t", [128, EMB], BF16),
        ("ident", [128, 128], BF16),
    ]:
        P[name] = nc.declare_dram_parameter(name, shape, dt, isOutput=False)
    out_ext = nc.declare_dram_parameter("out", [NLOC, EMB], F32, isOutput=True)

    # ---- internal DRAM
    shared = "Shared" if int(os.environ.get("K_SHARED", "1")) else "Local"
    xl0_loc = nc.dram_tensor("xl0_loc", [NLOC, FEAT0], BF16)
    xr0_loc = nc.dram_tensor("xr0_loc", [NLOC, FEAT0], BF16)
    xl0_full = nc.dram_tensor("xl0_full", [N, FEAT0], BF16, addr_space=shared)
    xl1_loc = nc.dram_tensor("xl1_loc", [NLOC, 128], BF16)
    xr1_loc = nc.dram_tensor("xr1_loc", [NLOC, EMB], BF16)
    xl1_full = nc.dram_tensor("xl1_full", [N, 128], BF16, addr_space=shared)

    rows_of = lambda b: min(128, NLOC - b * 128)

    with tile.TileContext(nc) as tc:
        with (
            tc.tile_pool(name="cst", bufs=1) as cst,
            tc.tile_pool(name="sb", bufs=1) as sb,
            tc.tile_pool(name="ps", bufs=1, space="PSUM") as ps,
        ):
            # ---- load constants
            C = {}
            for name in ["Wp", "bp_t", "Wl0", "Wr0", "bl0_t", "br0_t",
                         "W1h", "bpk1_t", "att0_t", "att1_t", "bga0_t",
                         "bga1_t", "g0_t", "be0_t", "g1_t", "be1_t", "ident"]:
                t = cst.tile(list(P[name].shape), P[name].dtype, tag=name)
                nc.sync.dma_start(t[:], P[name][:])
                C[name] = t
            mask_sb = cst.tile([128, TOTCH * 128], FP8, tag="masks")
            nc.sync.dma_start(mask_sb[:], P["masks"][:])
            maskT_sb = cst.tile([128, TOTCH * 128], FP8, tag="masksT")
            nc.sync.dma_start(maskT_sb[:], P["masksT"][:])
            eshift = cst.tile([128, 1], F32, tag="eshift")
            nc.vector.memset(eshift[:], EXP_SHIFT)

            def elu_f32(dst_ap, src_ap, shape, tag):
                """dst = elu(src)."""
                r = sb.tile(shape, F32, tag=f"{tag}_r")
                m = sb.tile(shape, F32, tag=f"{tag}_m")
                ep = sb.tile(shape, F32, tag=f"{tag}_e")
                nc.vector.tensor_scalar_max(r[:], src_ap, 0.0)
                nc.vector.tensor_scalar_min(m[:], src_ap, 0.0)
                nc.scalar.activation(ep[:], m[:], AF.Exp)
                nc.vector.scalar_tensor_tensor(dst_ap, r[:], -1.0, ep[:],
                                               op0=OP.add, op1=OP.add)

            def layer_norm(dst_ap, src_ap, width, g_t, be_t, tag):
                """dst = LN(src) * g + be  (rsqrt via exp(-0.5*ln))."""
                mu = sb.tile([128, 1], F32, tag=f"{tag}_mu")
                xc = sb.tile([128, width], F32, tag=f"{tag}_xc")
                sq = sb.tile([128, width], F32, tag=f"{tag}_sq")
                var = sb.tile([128, 1], F32, tag=f"{tag}_v")
                st = sb.tile([128, 1], F32, tag=f"{tag}_s")
                nc.vector.tensor_reduce(mu[:], src_ap, axis=mybir.AxisListType.X,
                                        op=OP.add)
                nc.vector.tensor_scalar_mul(mu[:], mu[:], 1.0 / width)
                nc.vector.tensor_scalar_sub(xc[:], src_ap, mu[:])
                nc.vector.scalar_tensor_tensor(sq[:], xc[:], 0.0, xc[:],
                                               op0=OP.add, op1=OP.mult,
                                               accum_out=var[:])
                nc.vector.tensor_scalar(st[:], var[:], 1.0 / width, LN_EPS,
                                        op0=OP.mult, op1=OP.add)
                # rsqrt(st) on DVE only (avoids Ln/Sqrt act-table reloads):
                # seed y0 = 2/(1+st) <= rsqrt(st), then Newton iterations
                y = sb.tile([128, 1], F32, tag=f"{tag}_y")
                q = sb.tile([128, 1], F32, tag=f"{tag}_q")
                nc.vector.tensor_scalar(y[:], st[:], 0.5, 0.5,
                                        op0=OP.mult, op1=OP.add)
                nc.vector.reciprocal(y[:], y[:])
                for _it in range(5):
                    nc.vector.tensor_tensor(q[:], y[:], y[:], OP.mult)
                    nc.vector.tensor_tensor(q[:], q[:], st[:], OP.mult)
                    nc.vector.tensor_scalar(q[:], q[:], -0.5, 1.5,
                                            op0=OP.mult, op1=OP.add)
                    nc.vector.tensor_tensor(y[:], y[:], q[:], OP.mult)
                nc.vector.tensor_scalar_mul(xc[:], xc[:], y[:])
                nc.vector.tensor_tensor(xc[:], xc[:], g_t[:], OP.mult)
                nc.vector.tensor_tensor(dst_ap, xc[:], be_t[:], OP.add)

            # ================= phase 1: layer-0 tables =================
            for ch in range(NBLK):
                rows = rows_of(ch)
                xt = sb.tile([128, 128], BF16, tag="p1_xt", bufs=2)
                nc.sync.dma_start(xt[:, :rows], P["xT"][:, ch * 128:ch * 128 + rows])
                ph = ps.tile([128, HID], F32, tag="px")
                nc.tensor.matmul(ph[:], xt[:], C["Wp"][:], start=True, stop=True)
                hb = sb.tile([128, HID], F32, tag="p1_hb")
                nc.vector.tensor_tensor(hb[:], ph[:], C["bp_t"][:], OP.add)
                h = sb.tile([128, HID], BF16, tag="p1_hbf")
                elu_f32(h[:], hb[:], [128, HID], "el")
                pt = ps.tile([128, 128], BF16, tag="pt1")
                nc.tensor.transpose(pt[0:HID, :], h[:], C["ident"][:])
                hT = sb.tile([HID, 128], BF16, tag="p1_hT")
                nc.scalar.copy(hT[:], pt[0:HID, :])
                pu = ps.tile([128, 512], F32, tag="pu", bufs=2)
                nc.tensor.matmul(pu[:, 0:FEAT0], hT[:], C["Wl0"][:],
                                 start=True, stop=True)
                nc.tensor.matmul(pu[:, FEAT0:2 * FEAT0], hT[:], C["Wr0"][:],
                                 start=True, stop=True)
                tabl = sb.tile([128, FEAT0], BF16, tag="p1_tabl")
                nc.vector.tensor_tensor(tabl[:], pu[:, 0:FEAT0], C["bl0_t"][:],
                                        OP.add)
                nc.sync.dma_start(xl0_loc[ch * 128:ch * 128 + rows, :],
                                  tabl[:rows, :])
                tabr = sb.tile([128, FEAT0], BF16, tag="p1_tabr")
                nc.vector.tensor_tensor(tabr[:], pu[:, FEAT0:2 * FEAT0],
                                        C["br0_t"][:], OP.add)
                nc.sync.dma_start(xr0_loc[ch * 128:ch * 128 + rows, :],
                                  tabr[:rows, :])

            # ================= AllGather layer-0 xl table ==============
            if int(os.environ.get("K_ST", "4")) >= 2:
                nc.gpsimd.collective_compute(
                    "AllGather", OP.bypass,
                    replica_groups=[list(range(NCORES))],
                    ins=[xl0_loc[:]], outs=[xl0_full[:]])

            # ================= per-layer edge phase ====================
            def edge_layer(layer):
                if layer == 0:
                    W, heads = FEAT0, HEADS
                    table, telem = xl0_full, FEAT0
                    xr_loc, att_t = xr0_loc, C["att0_t"]
                    grp = 2          # chunks per PSUM group (2*256 f32 = 2KB)
                else:
                    W, heads = EMB, 1
                    table, telem = xl1_full, 128
                    xr_loc, att_t = xr1_loc, C["att1_t"]
                    grp = 16         # 16*32 f32 = 2KB
                hd = W // heads

                # first-use memset so stale gather lanes can't hold NaN
                if layer == 0:
                    for _ in range(2):
                        xlg0 = sb.tile([128, nch, FEAT0], BF16,
                                       tag="xlg", bufs=2)
                        nc.vector.memset(xlg0[:], 0.0)

                tg = f"E{layer}"
                coll_t = sb.tile([128, NBLK, FEAT0], BF16, tag="coll")
                scoll_t = sb.tile([128, NBLK, HEADS], F32, tag="scoll")
                coll = coll_t[:, :, 0:W]
                scoll = scoll_t[:, :, 0:heads]

                for b in range(NBLK):
                    rows = rows_of(b)
                    nb = ncb[b]
                    si = sb.tile([128, eb // 16], I16, tag=f"{tg}_si", bufs=2)
                    nc.sync.dma_start(si[:], P["sidx"][b])
                    xrb = sb.tile([128, W], BF16, tag=f"{tg}_xrb")
                    nc.sync.dma_start(xrb[:rows, :],
                                      xr_loc[b * 128:b * 128 + rows, 0:W])
                    xlg_t = sb.tile([128, nch, FEAT0], BF16, tag="xlg",
                                    bufs=2)
                    xlg = (xlg_t[:] if layer == 0 else
                           xlg_t[:].rearrange("p n k -> p (n k)").rearrange(
                               "p (m j) -> p m j", j=128))
                    GMAX = int(os.environ.get("K_GMAX", "1024")) // 128
                    q = b % NQ
                    for c0 in range(0, nb, GMAX):
                        cn = min(GMAX, nb - c0)
                        nc.gpsimd.dma_gather(
                            xlg[:, c0:c0 + cn, 0:telem], table[:, 0:telem],
                            si[:, c0 * 8:(c0 + cn) * 8], cn * 128, cn * 128,
                            telem, elem_step=telem, queue_num=q)
                        q = (q + 1) % NQ

                    EL = int(os.environ.get("K_EL", "5"))
                    if EL < 2:
                        continue
                    # u = xl + xr  (PE: ident-inject + one-hot expansion)
                    # z = lrelu(u) (Scalar, PSUM -> SBUF); the same tile is
                    # later overwritten with wa = [a*xl | a]
                    zwa = sb.tile([128, nch, W + heads], BF16, tag=f"{tg}_zw")
                    z = zwa[:, :, 0:W]
                    ngrp = (nb + grp - 1) // grp
                    for g in range(ngrp):
                        gn = min(grp, nb - g * grp)
                        pu = ps.tile([128, 512], F32, tag="pu", bufs=2)
                        # batched ident-inject: one matmul over the group
                        nc.tensor.matmul(
                            pu[:, 0:gn * W].rearrange("p (n k) -> p n k", k=W),
                            C["ident"][:], xlg[:, g * grp:g * grp + gn, 0:W],
                            start=True, stop=False)
                        for j in range(gn):
                            chk = g * grp + j
                            mT = maskT_sb[:, (offs[b] + chk) * 128:
                                          (offs[b] + chk + 1) * 128]
                            nc.tensor.matmul(pu[:, j * W:(j + 1) * W], mT,
                                             xrb[:, 0:W],
                                             start=False, stop=(j == gn - 1))
                        zv = z[:, g * grp:g * grp + gn, :]
                        lt = sb.tile([128, 512], BF16, tag=f"{tg}_lt", bufs=2)
                        nc.scalar.activation(
                            lt[:, 0:gn * W].rearrange("p (n k) -> p n k", k=W),
                            pu[:, 0:gn * W].rearrange("p (n k) -> p n k", k=W),
                            AF.Relu, scale=A_LR)
                        nc.vector.scalar_tensor_tensor(
                            zv,
                            pu[:, 0:gn * W].rearrange("p (n k) -> p n k", k=W),
                            B_LR,
                            lt[:, 0:gn * W].rearrange("p (n k) -> p n k", k=W),
                            op0=OP.mult, op1=OP.add)

                    if EL < 3:
                        continue
                    # e = per-head att dot; a = exp(e + EXP_SHIFT)
                    z3 = zwa[:, 0:nb, 0:W]
                    nc.vector.tensor_tensor(z3, z3, _mid_bcast(att_t[:], nb),
                                            OP.mult)
                    e = sb.tile([128, nch, heads], F32, tag=f"{tg}_e")
                    z4 = z3.rearrange("p n (h k) -> p n h k", k=hd)
                    nc.vector.tensor_reduce(e[:, 0:nb, :], z4,
                                            axis=mybir.AxisListType.X, op=OP.add)
                    a = sb.tile([128, nch, heads], BF16, tag=f"{tg}_a")
                    nc.scalar.activation(a[:, 0:nb, :], e[:, 0:nb, :], AF.Exp,
                                         bias=eshift[:])

                    if EL < 4:
                        continue
                    # wa = [a*xl | a]  (overwrites the z tile)
                    wa = zwa
                    wa_x = wa[:, 0:nb, 0:W].rearrange("p n (h k) -> p n h k",
                                                      k=hd)
                    a4 = a[:, 0:nb, :].rearrange("p n (h o) -> p n h o", o=1)
                    nc.vector.tensor_tensor(
                        wa_x, xlg[:, 0:nb, 0:W].rearrange(
                            "p n (h k) -> p n h k", k=hd),
                        _bcast(a4, [hd])[:, :, :, 0], OP.mult)
                    nc.vector.tensor_copy(wa[:, 0:nb, W:W + heads],
                                          a[:, 0:nb, :])

                    # masked-matmul aggregation (messages + softmax denoms)
                    po = ps.tile([128, W + heads], F32, tag="po", bufs=2)
                    for chk in range(nb):
                        mk = mask_sb[:, (offs[b] + chk) * 128:
                                     (offs[b] + chk + 1) * 128]
                        nc.tensor.matmul(po[:], mk, wa[:, chk, :],
                                         start=(chk == 0), stop=(chk == nb - 1))

                    if EL < 5:
                        continue
                    # stash block results; epilogue is batched per layer
                    nc.vector.tensor_copy(coll[:, b, :], po[:, 0:W])
                    nc.vector.tensor_copy(scoll[:, b, :], po[:, W:W + heads])

                # ======== batched epilogue over all blocks ========
                if EL < 5:
                    return
                gb = C["bga0_t"] if layer == 0 else C["bga1_t"]
                g_t = C["g0_t"] if layer == 0 else C["g1_t"]
                be_t = C["be0_t"] if layer == 0 else C["be1_t"]
                # alpha divide: coll *= 1/(s+eps), per head
                nc.vector.tensor_scalar_add(scoll, scoll, SM_EPS)
                nc.vector.reciprocal(scoll, scoll)
                c4 = coll.rearrange("p b (h k) -> p b h k", k=hd)
                nc.vector.tensor_tensor(c4, c4, _bcast(scoll, [hd]),
                                        OP.mult)
                nc.vector.tensor_tensor(coll, coll,
                                        _mid_bcast(gb[:], NBLK), OP.add)
                # layer norm over the last W features of each (p, b) row
                mu = sb.tile([128, NBLK], F32, tag=f"{tg}_mu")
                nc.vector.tensor_reduce(mu[:], coll,
                                        axis=mybir.AxisListType.X, op=OP.add)
                nc.vector.tensor_scalar_mul(mu[:], mu[:], 1.0 / W)
                xc_t = sb.tile([128, NBLK, FEAT0], BF16, tag="xc")
                xc = xc_t[:, :, 0:W]
                nc.vector.tensor_tensor(xc, coll,
                                        _bcast(mu[:], [W]), OP.subtract)
                nc.vector.scalar_tensor_tensor(coll, xc, 0.0, xc,
                                               op0=OP.add, op1=OP.mult)
                var = sb.tile([128, NBLK], F32, tag=f"{tg}_var")
                nc.vector.tensor_reduce(var[:], coll,
                                        axis=mybir.AxisListType.X, op=OP.add)
                st = sb.tile([128, NBLK], F32, tag=f"{tg}_st")
                nc.vector.tensor_scalar(st[:], var[:], 1.0 / W, LN_EPS,
                                        op0=OP.mult, op1=OP.add)
                # rsqrt(st) via reciprocal seed + Newton (all-DVE, no tables)
                y = sb.tile([128, NBLK], F32, tag=f"{tg}_y")
                q = sb.tile([128, NBLK], F32, tag=f"{tg}_q")
                nc.vector.tensor_scalar(y[:], st[:], 0.5, 0.5,
                                        op0=OP.mult, op1=OP.add)
                nc.vector.reciprocal(y[:], y[:])
                for _it in range(5):
                    nc.vector.scalar_tensor_tensor(q[:], y[:], 0.0, y[:],
                                                   op0=OP.add, op1=OP.mult)
                    nc.vector.scalar_tensor_tensor(q[:], q[:], -0.5, st[:],
                                                   op0=OP.mult, op1=OP.mult)
                    nc.vector.scalar_tensor_tensor(y[:], q[:], 1.5, y[:],
                                                   op0=OP.add, op1=OP.mult)
                nc.vector.tensor_tensor(xc, xc,
                                        _bcast(y[:], [W]), OP.mult)
                nc.vector.tensor_tensor(xc, xc,
                                        _mid_bcast(g_t[:], NBLK), OP.mult)
                if layer == 0:
                    nc.vector.tensor_tensor(xc, xc,
                                            _mid_bcast(be_t[:], NBLK), OP.add)
                    # elu -> h1 (into coll): coll=relu(xc); xc=exp(min(xc,0))
                    nc.vector.tensor_scalar_max(coll, xc, 0.0)
                    nc.vector.tensor_scalar_min(xc, xc, 0.0)
                    nc.scalar.activation(xc, xc, AF.Exp)
                    nc.vector.scalar_tensor_tensor(coll, coll, -1.0,
                                                   xc, op0=OP.add,
                                                   op1=OP.add)
                    # layer-1 table rows per block
                    for b in range(NBLK):
                        rows = rows_of(b)
                        px = ps.tile([128, HID], F32, tag="px")
                        for half in range(2):
                            pt1 = ps.tile([128, 128], BF16, tag="pt1")
                            nc.tensor.transpose(
                                pt1[:],
                                coll_t[:, b, half * 128:(half + 1) * 128],
                                C["ident"][:])
                            hT1 = sb.tile([128, 128], BF16, tag=f"{tg}_hT1")
                            nc.scalar.copy(hT1[:], pt1[:])
                            nc.tensor.matmul(
                                px[:], hT1[:],
                                C["W1h"][:, half * 2 * EMB:(half + 1) * 2 * EMB],
                                start=(half == 0), stop=(half == 1))
                        tb1 = sb.tile([128, 2 * EMB], BF16, tag=f"{tg}_tb1")
                        nc.vector.tensor_tensor(tb1[:], px[:], C["bpk1_t"][:],
                                                OP.add)
                        nc.sync.dma_start(
                            xl1_loc[b * 128:b * 128 + rows, 0:EMB],
                            tb1[:rows, 0:EMB])
                        nc.sync.dma_start(
                            xr1_loc[b * 128:b * 128 + rows, :],
                            tb1[:rows, EMB:2 * EMB])
                else:
                    nc.vector.tensor_tensor(xc, xc,
                                            _mid_bcast(be_t[:], NBLK), OP.add)
                    nfull = (NLOC // 128) * 128
                    nc.gpsimd.dma_start(
                        out_ext[0:nfull, :].rearrange("(b p) w -> p b w",
                                                      p=128),
                        xc_t[:, 0:NLOC // 128, 0:W])
                    lb = NLOC // 128
                    nc.gpsimd.dma_start(out_ext[nfull:NLOC, :],
                                        xc_t[0:NLOC - nfull, lb, 0:W])

            ST = int(os.environ.get("K_ST", "4"))
            if ST >= 3:
                edge_layer(0)
            if ST >= 4:
                nc.gpsimd.collective_compute(
                    "AllGather", OP.bypass,
                    replica_groups=[list(range(NCORES))],
                    ins=[xl1_loc[:]], outs=[xl1_full[:]])
                edge_layer(1)

    nc.compile()
    return nc


def kernel(**inputs):
    xT, sidx, masks, masksT, consts, eb, nch, ncb, offs = _host_prep(**inputs)
    nc = _build(eb, nch, ncb, offs)
    in_maps = []
    for c in range(NCORES):
        m = {
            "xT": np.ascontiguousarray(xT[:, c * NLOC:(c + 1) * NLOC]),
            "sidx": sidx[c], "masks": masks[c], "masksT": masksT[c],
        }
        m.update(consts)
        in_maps.append(m)
    trace = bool(int(os.environ.get("K_TRACE", "0")))
    res = run_bass_kernel_spmd(nc, in_maps, list(range(NCORES)), trace=trace)
    global LAST_EXEC_NS, LAST_RESULT
    LAST_EXEC_NS = res.exec_time_ns
    LAST_RESULT = res
    out = np.concatenate([np.asarray(res.results[c]["out"]) for c in range(NCORES)], 0)
    return out.astype(np.float32)
